# revision 6
# baseline (speedup 1.0000x reference)
"""Trainium2 Bass kernel for nn_Attention (B=4, C=256, L=2048, H=8 heads, D=64).

Sharding: head-parallel across 8 NeuronCores (1 head per core). Each core:
  - projects its head's Q/K/V from the full input x (channels-first),
  - runs attention in the S^T (keys-on-partitions) layout so softmax's
    denominator comes free from an appended ones-column in the P^T @ V^T
    matmul (M=65),
  - normalizes + casts its head output to fp16,
  - one AllToAll redistributes head outputs so each core owns all 8 heads
    for 1/8 of the (b, l) columns,
  - each core applies w_out + bias on its column shard.
Host reassembles the 8 column shards into the full [B, C, L] output.

dtypes: QKV projection and S^T matmuls run as float32r (full-rate fp32 on
the PE), P^T/V^T/out-projection run fp16, accumulation is always fp32 PSUM.
"""

import os
import sys

import numpy as np

sys.path.insert(0, "/opt/trn_rl_repo")

import concourse.bass as bass  # noqa: E402
import concourse.bacc as bacc  # noqa: E402
import concourse.tile as tile  # noqa: E402
import concourse.mybir as mybir  # noqa: E402
import concourse.bass_utils as bass_utils  # noqa: E402
from concourse.bass_interp import get_hw_module  # noqa: E402

B, C, L = 4, 256, 2048
H, D = 8, 64
NCORES = 8
N = B * L                # 8192 flattened (b, l) columns
NSH = N // NCORES        # 1024 columns per core in the output shard
NBLK = 512               # matmul free-dim block
F32 = mybir.dt.float32
F32R = mybir.dt.float32r
F16 = mybir.dt.float16
AF = mybir.ActivationFunctionType

_CACHE = {}


def _build():
    nc = bacc.Bacc("TRN2", target_bir_lowering=False, debug=False,
                   num_devices=NCORES)

    x_t = nc.dram_tensor("x_t", [2, 128, N], F16, kind="ExternalInput")
    wq_p = nc.dram_tensor("wq_p", [128, 128], F16, kind="ExternalInput")
    wk_p = nc.dram_tensor("wk_p", [128, 128], F16, kind="ExternalInput")
    wv_p = nc.dram_tensor("wv_p", [128, 128], F16, kind="ExternalInput")
    wo_p = nc.dram_tensor("wo_p", [128, 4, 256], F16, kind="ExternalInput")
    bias2 = nc.dram_tensor("bias2", [128, 2], F32, kind="ExternalInput")
    out = nc.dram_tensor("out", [2, 128, NSH], F32, kind="ExternalOutput")

    ident_d = nc.inline_tensor(np.eye(64, dtype=np.float16), name="ident64")

    with tile.TileContext(nc) as tc:
        with (
            tc.tile_pool(name="const", bufs=1) as cpool,
            tc.tile_pool(name="qk", bufs=2) as qkpool,
            tc.tile_pool(name="vt", bufs=2) as vtpool,
            tc.tile_pool(name="pt", bufs=3) as ptpool,
            tc.tile_pool(name="small", bufs=2) as spool,
            tc.tile_pool(name="psA", bufs=2, space="PSUM") as psA,
            tc.tile_pool(name="psO", bufs=2, space="PSUM") as psO,
            tc.tile_pool(name="psP", bufs=2, space="PSUM") as psP,
            tc.tile_pool(name="dram", bufs=1, space="DRAM") as dpool,
        ):
            # ---- constants / weights into SBUF ----
            x_sb = cpool.tile([128, 2 * N], F16, name="x_sb")
            for ch in range(2):
                for s in range(8):
                    nc.sync.dma_start(
                        x_sb[:, ch * N + s * 1024:ch * N + (s + 1) * 1024],
                        x_t[ch, :, s * 1024:(s + 1) * 1024],
                    )
            wq_sb = cpool.tile([128, 128], F16, name="wq_sb")
            wk_sb = cpool.tile([128, 128], F16, name="wk_sb")
            wv_sb = cpool.tile([128, 128], F16, name="wv_sb")
            wo_sb = cpool.tile([128, 1024], F16, name="wo_sb")
            bias_sb = cpool.tile([128, 2], F32, name="bias_sb")
            ident_sb = cpool.tile([64, 64], F16, name="ident_sb")
            nc.sync.dma_start(wq_sb[:], wq_p[:])
            nc.sync.dma_start(wk_sb[:], wk_p[:])
            nc.sync.dma_start(wv_sb[:], wv_p[:])
            nc.sync.dma_start(wo_sb.rearrange("p (c o) -> p c o", c=4), wo_p[:])
            nc.sync.dma_start(bias_sb[:], bias2[:])
            nc.sync.dma_start(ident_sb[:], ident_d[:])

            bounce_in = dpool.tile([NCORES, 64, NSH], F16, name="bounce_in")
            bounce_out = dpool.tile([NCORES, 64, NSH], F16, name="bounce_out")

            def proj_pair(ps, w_sb, nb_lo, nb_hi, b):
                """Project x columns into psum: strip 0 <- block nb_lo,
                strip 64 <- block nb_hi (M=64 each, K=256 via 2 halves)."""
                for strip, nb in ((0, nb_lo), (64, nb_hi)):
                    o_ap = ps[strip:strip + 64, :]
                    for ch in range(2):
                        col0 = ch * N + b * L + nb * NBLK
                        nc.tensor.matmul(
                            o_ap,
                            w_sb[:, ch * 64:(ch + 1) * 64],
                            x_sb[:, col0:col0 + NBLK],
                            start=(ch == 0), stop=(ch == 1),
                            tile_position=(0, strip),
                        )

            for b in range(B):
                # ---- Q/K projections, duplicated across partition halves ----
                qd = qkpool.tile([128, L], F16, name="qd", tag="qd")
                kd = qkpool.tile([128, L], F16, name="kd", tag="kd")
                for nb in range(4):
                    psq = psP.tile([128, NBLK], F32, name="psq", tag="psp")
                    proj_pair(psq, wq_sb, nb, nb, b)
                    nc.vector.tensor_copy(qd[:, nb * NBLK:(nb + 1) * NBLK], psq[:])
                    psk = psP.tile([128, NBLK], F32, name="psk", tag="psp")
                    proj_pair(psk, wk_sb, nb, nb, b)
                    nc.vector.tensor_copy(kd[:, nb * NBLK:(nb + 1) * NBLK], psk[:])
                # ---- V projection (channels-first), 2 blocks per psum ----
                vc = vtpool.tile([64, L], F16, name="vc", tag="vc")
                for nbp in range(2):
                    psv = psP.tile([128, NBLK], F32, name="psv", tag="psp")
                    proj_pair(psv, wv_sb, 2 * nbp, 2 * nbp + 1, b)
                    nc.vector.tensor_copy(
                        vc[:, (2 * nbp) * NBLK:(2 * nbp + 1) * NBLK], psv[0:64, :])
                    nc.vector.tensor_copy(
                        vc[:, (2 * nbp + 1) * NBLK:(2 * nbp + 2) * NBLK],
                        psv[64:128, :])
                # ---- V^T (+ ones column) via PE transpose ----
                vt = vtpool.tile([128, 16 * 65], F16, name="vt", tag="vt")
                vt3 = vt.rearrange("p (j e) -> p j e", e=65)
                nc.vector.memset(vt3[:, :, 64], 1.0)
                for jt in range(8):
                    pst = psP.tile([128, 128], F16, name="pst", tag="psp")
                    nc.tensor.transpose(
                        pst[:, 0:64],
                        vc[:, (2 * jt) * 128:(2 * jt + 1) * 128], ident_sb[:])
                    nc.tensor.transpose(
                        pst[:, 64:128],
                        vc[:, (2 * jt + 1) * 128:(2 * jt + 2) * 128], ident_sb[:])
                    nc.vector.tensor_copy(
                        vt3[:, 2 * jt:2 * jt + 2, 0:64],
                        pst.rearrange("p (j e) -> p j e", e=64))
                # ---- attention over i-blocks ----
                for ib in range(4):
                    pso = psO.tile([65, NBLK], F32, name="pso", tag="pso")
                    for jp in range(8):
                        jA, jB = 2 * jp, 2 * jp + 1
                        pss = psA.tile([128, 2 * NBLK], F32, name="pss", tag="pss")
                        nc.tensor.matmul(
                            pss[:, 0:NBLK],
                            kd[0:64, jA * 128:(jA + 1) * 128],
                            qd[0:64, ib * NBLK:(ib + 1) * NBLK],
                            start=True, stop=True, tile_position=(0, 0))
                        nc.tensor.matmul(
                            pss[:, NBLK:2 * NBLK],
                            kd[64:128, jB * 128:(jB + 1) * 128],
                            qd[64:128, ib * NBLK:(ib + 1) * NBLK],
                            start=True, stop=True, tile_position=(64, 0))
                        pt = ptpool.tile([128, 2 * NBLK], F16, name="pt", tag="pt")
                        nc.scalar.activation(pt[:], pss[:], AF.Exp)
                        nc.tensor.matmul(
                            pso[:], vt3[:, jA, :], pt[:, 0:NBLK],
                            start=(jp == 0), stop=False)
                        nc.tensor.matmul(
                            pso[:], vt3[:, jB, :], pt[:, NBLK:2 * NBLK],
                            start=False, stop=(jp == 7))
                    recip = spool.tile([1, NBLK], F32, name="recip", tag="recip")
                    nc.vector.reciprocal(recip[:], pso[64:65, :])
                    bc = spool.tile([64, NBLK], F32, name="bc", tag="bc")
                    nc.gpsimd.partition_broadcast(bc[:], recip[:])
                    on = spool.tile([64, NBLK], F16, name="on", tag="on")
                    nc.vector.tensor_mul(on[:], pso[0:64, :], bc[:])
                    n0 = b * L + ib * NBLK
                    nc.sync.dma_start(
                        bounce_in[n0 // NSH, :, (n0 % NSH):(n0 % NSH) + NBLK],
                        on[:])

            # ---- redistribute: head-sharding -> column-sharding ----
            nc.gpsimd.collective_compute(
                "AllToAll", mybir.AluOpType.bypass,
                replica_groups=[list(range(NCORES))],
                ins=[bounce_in.opt()], outs=[bounce_out.opt()])

            gh = cpool.tile([128, 4096], F16, name="gh")
            for hc in range(4):
                for hp in range(2):
                    nc.sync.dma_start(
                        gh[hp * 64:(hp + 1) * 64, hc * 1024:(hc + 1) * 1024],
                        bounce_out[hc * 2 + hp, :, :])

            # ---- output projection + bias on this core's column shard ----
            for oh in range(2):
                for lb in range(2):
                    psy = psP.tile([128, NBLK], F32, name="psy", tag="psp")
                    for c in range(4):
                        nc.tensor.matmul(
                            psy[:],
                            wo_sb[:, c * 256 + oh * 128:c * 256 + (oh + 1) * 128],
                            gh[:, c * 1024 + lb * NBLK:c * 1024 + (lb + 1) * NBLK],
                            start=(c == 0), stop=(c == 3))
                    y = spool.tile([128, NBLK], F32, name="y", tag="y")
                    nc.vector.tensor_scalar_add(y[:], psy[:], bias_sb[:, oh:oh + 1])
                    nc.sync.dma_start(out[oh, :, lb * NBLK:(lb + 1) * NBLK], y[:])

    nc.compile()
    nc.m = get_hw_module(nc.m)
    return nc


def _prep_in_maps(x, w_qkv, w_out, b_out):
    scale = float(D) ** -0.5
    x = np.asarray(x, np.float32)
    w_qkv = np.asarray(w_qkv, np.float32)
    w_out = np.asarray(w_out, np.float32)
    b_out = np.asarray(b_out, np.float32)

    x_in = np.ascontiguousarray(
        x.transpose(1, 0, 2).reshape(C, N).reshape(2, 128, N)).astype(np.float16)
    wq = w_qkv[0:512].reshape(H, D, C) * scale
    wk = w_qkv[512:1024].reshape(H, D, C)
    wv = w_qkv[1024:1536].reshape(H, D, C)

    def pack_w(w):  # [64, 256] -> [128, (ch o)] with c = ch*128 + p
        return np.ascontiguousarray(
            w.T.reshape(2, 128, 64).transpose(1, 0, 2).reshape(128, 128)
        ).astype(np.float16)

    wo_p = np.ascontiguousarray(
        w_out.T.reshape(4, 2, 64, 256).transpose(1, 2, 0, 3).reshape(128, 4, 256)
    ).astype(np.float16)
    bias2 = np.ascontiguousarray(b_out.reshape(2, 128).T)

    in_maps = []
    for h in range(NCORES):
        in_maps.append({
            "x_t": x_in,
            "wq_p": pack_w(wq[h]),
            "wk_p": pack_w(wk[h]),
            "wv_p": pack_w(wv[h]),
            "wo_p": wo_p,
            "bias2": bias2,
        })
    return in_maps


def _run(inputs, trace=False):
    if "nc" not in _CACHE:
        _CACHE["nc"] = _build()
    nc = _CACHE["nc"]
    in_maps = _prep_in_maps(**inputs)
    res = bass_utils.run_bass_kernel_spmd(
        nc, in_maps, core_ids=list(range(NCORES)), trace=trace)
    y = np.empty((B, C, L), np.float32)
    for j in range(NCORES):
        shard = res.results[j]["out"].reshape(C, NSH)
        y[j // 2, :, (j % 2) * NSH:(j % 2 + 1) * NSH] = shard
    return y, res


def kernel(x, w_qkv, w_out, b_out):
    y, _ = _run(dict(x=x, w_qkv=w_qkv, w_out=w_out, b_out=b_out), trace=False)
    return y
